# revision 27
# baseline (speedup 1.0000x reference)
"""EventDrivenODECell Trainium2 kernel.

Math (reference semantics):
  dt = (t_end - t_start)/5
  5 Euler steps: h += dt * (W3 tanh(W2 tanh(W1a h + [bd1 + W1b te(t)])) + bd3)
    where te(t) depends only on the scalar t -> folded on host into a
    per-step bias  b1s = bd1 + W1b @ te(t_s);  dt folded into W3/bd3.
  event: out = h + sigmoid(Wg ef + bg) * (We2 relu(We1h h + We1e ef + be1) + be2)

Device layout: feature-major activations [feat, batch]; batch sharded 8 ways
(8192 rows/core) processed in 8 column-chunks of 1024 rows. PSUM tiles are
[128, 2, 512] (2 banks); each N=512 matmul targets one bank while ACT/DVE
drain the pair in one 1024-wide op. Matmuls are float16 (full PE rate, FWL
weight loads, exact fp32 PSUM accumulation) except we2 which runs fp8e4
DoubleRow (K=256 in one instruction): its input u1 is written as e4m3 by the
relu and We2 is pre-scaled x16 on host so no weight lands in e4m3's
subnormal range; the relu stores u1/16 (relu commutes with positive scale,
via ACT's free input scale) so the x16 cancels inside the matmul and the
downstream ops are unchanged. Host-simulated rel err: 4.2e-3.
fp8 for the ODE/W1/W2/W3 matmuls was measured (host sim) at 2.2-2.4e-2 rel
err per weight tensor -- the quantization error is coherent across the 5
Euler steps -- so those stay f16.

Issue order is software-pipelined per chunk (L1(c), L2(c-1), L3(c-2)) so the
psum drains alternate ACT/ACT/DVE every slot instead of bunching per layer
sweep, and half the event sigmoids are hoisted into ODE slots where ACT has
slack. The event pipeline (gate/u1/we2) is chunk-staggered the same way.
gates/tmp/stage/outT are f16 (halves SBUF and store traffic).
"""

import os
import sys

sys.path.insert(0, "/opt/trn_rl_repo")

import numpy as np
import ml_dtypes

import concourse.bacc as bacc
import concourse.mybir as mybir
import concourse.tile as tile
from concourse.bass_utils import run_bass_kernel_spmd

B = 65536
HID = 256
EVT = 64
TEMB = 32
NUM_STEPS = 5
N_CORES = 8
R = B // N_CORES          # rows per core
S = 512                   # matmul moving-dim / PSUM bank quantum
CHUNK = 1024              # rows per processing chunk (= 2 PSUM banks)
NS = CHUNK // S           # N-splits per chunk
N_CHUNKS = R // CHUNK     # 8

WE2_SCALE = 16.0          # keeps We2 (|w| <= 1/16) out of e4m3 subnormals

MODE = "f16+we2fp8"

f32 = mybir.dt.float32
f16 = mybir.dt.float16
f8e4 = mybir.dt.float8e4
E4NP = ml_dtypes.float8_e4m3

_CACHE = {}

# bias-pack column indices
COL_B1S = 0          # 0..4: per-step layer-1 bias
COL_B2 = 5
COL_B3 = 6
COL_BE1S = 7         # be1 / 16 (the relu writes u1/16; see ew())
COL_BE2 = 8
COL_BG = 9
N_BIAS_COLS = 10


def _build():
    wdt = f16
    nc = bacc.Bacc("TRN2", target_bir_lowering=False, debug=False,
                   num_devices=N_CORES)

    hT_d = nc.dram_tensor("hT", [HID, R], wdt, kind="ExternalInput")
    efT_d = nc.dram_tensor("efT", [EVT, R], wdt, kind="ExternalInput")
    w1_d = nc.dram_tensor("w1", [HID, HID], wdt, kind="ExternalInput")
    w2_d = nc.dram_tensor("w2", [HID, HID], wdt, kind="ExternalInput")
    w3_d = nc.dram_tensor("w3", [HID, HID], wdt, kind="ExternalInput")
    we1h_d = nc.dram_tensor("we1h", [HID, HID], wdt, kind="ExternalInput")
    we1e_d = nc.dram_tensor("we1e", [EVT, HID], wdt, kind="ExternalInput")
    we2dr_d = nc.dram_tensor("we2dr", [128, 2 * HID], f8e4,
                             kind="ExternalInput")
    wg_d = nc.dram_tensor("wg", [EVT, HID], wdt, kind="ExternalInput")
    biasp_d = nc.dram_tensor("biasp", [HID, N_BIAS_COLS], f32,
                             kind="ExternalInput")
    outT_d = nc.dram_tensor("outT", [HID, R], f16, kind="ExternalOutput")

    Tanh = mybir.ActivationFunctionType.Tanh
    Sigmoid = mybir.ActivationFunctionType.Sigmoid
    Relu = mybir.ActivationFunctionType.Relu
    Ident = mybir.ActivationFunctionType.Identity
    add = mybir.AluOpType.add
    mult = mybir.AluOpType.mult
    DR = mybir.MatmulPerfMode.DoubleRow

    with tile.TileContext(nc) as tc:
        with (
            tc.tile_pool(name="consts", bufs=1) as consts,
            tc.tile_pool(name="h", bufs=1) as h_pool,
            tc.tile_pool(name="z1", bufs=12) as z1_pool,
            tc.tile_pool(name="z2", bufs=12) as z2_pool,
            tc.tile_pool(name="gts", bufs=16) as g_pool,
            tc.tile_pool(name="tmp", bufs=6) as t_pool,
            tc.tile_pool(name="u1", bufs=6) as u_pool,
            tc.tile_pool(name="efc", bufs=8) as ef_pool,
            tc.tile_pool(name="stage", bufs=6) as stage_pool,
            tc.tile_pool(name="psum", bufs=4, space="PSUM") as psum_pool,
        ):
            # ---- constants / h tiles; DMA issue order matters (single
            # HWDGE queue serializes) so interleave with first-use order ----
            def load_w(d, name, kparts, kdim=128):
                ts = []
                for k in range(kparts):
                    t = consts.tile([kdim, HID], wdt, tag=f"{name}{k}",
                                    name=f"{name}{k}")
                    nc.sync.dma_start(t[:], d.ap()[k * kdim:(k + 1) * kdim, :])
                    ts.append(t)
                return ts

            h = [[h_pool.tile([128, NS, S], wdt, tag=f"h{c}_{m}",
                              name=f"h{c}_{m}")
                  for m in range(2)] for c in range(N_CHUNKS)]

            def load_h(c):
                for m in range(2):
                    nc.sync.dma_start(
                        h[c][m][:],
                        hT_d.ap()[m * 128:(m + 1) * 128,
                                  c * CHUNK:(c + 1) * CHUNK])

            w1 = load_w(w1_d, "w1", 2)
            load_h(0)
            load_h(1)
            biasp = []
            for m in range(2):
                t = consts.tile([128, N_BIAS_COLS], f32, tag=f"biasp{m}",
                                name=f"biasp{m}")
                nc.sync.dma_start(t[:], biasp_d.ap()[m * 128:(m + 1) * 128, :])
                biasp.append(t)
            w2 = load_w(w2_d, "w2", 2)
            load_h(2)
            w3 = load_w(w3_d, "w3", 2)
            load_h(3)
            we1h = load_w(we1h_d, "we1h", 2)
            load_h(4)
            load_h(5)
            # we2 DoubleRow weights: [k=128, group=2, 256] e4m3, x16
            we2dr = consts.tile([128, 2, HID], f8e4, tag="we2dr",
                                name="we2dr")
            nc.sync.dma_start(we2dr[:], we2dr_d.ap())

            # EVT-dim weights live in both partition halves so the two
            # m-half K=64 matmuls can run on distinct PE row groups.
            def load_evt_w(d, name):
                t = consts.tile([128, HID], wdt, tag=name, name=name)
                nc.sync.dma_start(t[0:EVT, :], d.ap())
                nc.sync.dma_start(t[EVT:128, :], d.ap())
                return t

            we1e = load_evt_w(we1e_d, "we1e")   # [128, 256], duplicated rows
            wg = load_evt_w(wg_d, "wg")
            load_h(6)
            load_h(7)
            # event features early: DMA is idle mid-kernel and the event
            # phase must not wait on loads
            efs = {}
            for c in range(N_CHUNKS):
                efc = ef_pool.tile([128, NS, S], wdt, tag="ef",
                                   name=f"ef{c}")
                for half in range(2):
                    nc.sync.dma_start(
                        efc[half * EVT:(half + 1) * EVT],
                        efT_d.ap()[:, c * CHUNK:(c + 1) * CHUNK])
                efs[c] = efc

            def bcol(m, col):
                return biasp[m][:, col:col + 1]

            # ---- PE warmup: dependency-free junk matmuls on the
            # framework const AP (materialized by the preamble TENSOR_LOADs,
            # so no memset/DMA dependency -- the PE starts the HAM ramp the
            # moment its queue opens, ~1.7us earlier than a memset-fed
            # tile). N=1 f32 matmuls keep the array draining ~continuously
            # at ~60-cycle issue spacing. ----
            one = nc.const_aps.tensor(1.0, (128, 1), f32)
            wps = psum_pool.tile([1, 1], f32, tag="ps", name="wps")
            for _ in range(56):
                nc.tensor.matmul(wps[:], one, one, start=True, stop=True)
            # prefetch both ACT function tables while ACT is idle
            wz = stage_pool.tile([128, 1], f16, tag="wz", name="wz")
            nc.scalar.activation(wz[:], one, Tanh)
            nc.scalar.activation(wz[:], one, Sigmoid)

            def mm_chunk(ps, win, x, m, kparts=2, extra=None):
                """ps [128,NS,S] (PSUM) += win[k][:, m-blk].T @ x[k] per
                N-split; optional extra=(w_evt, ef_tile) accumulated last."""
                n_acc = kparts + (1 if extra is not None else 0)
                for k in range(kparts):
                    wblk = win[k][:, m * 128:(m + 1) * 128]
                    for j in range(NS):
                        nc.tensor.matmul(ps[:, j], wblk, x[k][:, j],
                                         start=(k == 0),
                                         stop=(k == n_acc - 1))
                if extra is not None:
                    ew, ex = extra
                    eblk = ew[:, m * 128:(m + 1) * 128]
                    for j in range(NS):
                        nc.tensor.matmul(ps[:, j], eblk, ex[:, j],
                                         start=False, stop=True)

            def dense(out_pool, win, x_tiles, bias_col, act, out_dt=wdt):
                """[2 x [128,NS,S]] tiles: act(win.T @ x + bias)."""
                outs = []
                for m in range(2):
                    ps = psum_pool.tile([128, NS, S], f32, tag="ps",
                                        name=f"ps{m}")
                    mm_chunk(ps, win, x_tiles, m)
                    o = out_pool.tile([128, NS, S], out_dt, tag="z",
                                      name=f"z{m}")
                    nc.scalar.activation(o[:], ps[:], act,
                                         bias=bcol(m, bias_col))
                    outs.append(o)
                return outs

            z1s = {}
            z2s = {}
            gates = {}
            u1s = {}

            def l1(s, c):
                z1s[c] = dense(z1_pool, w1, h[c], COL_B1S + s, Tanh)

            def l2(s, c):
                z2s[c] = dense(z2_pool, w2, z1s[c], COL_B2, Tanh)

            def l3(s, c):
                for m in range(2):
                    ps = psum_pool.tile([128, NS, S], f32, tag="ps",
                                        name=f"ps3{m}")
                    mm_chunk(ps, w3, z2s[c], m)
                    # h += (psum + b3)  (rounds h to f16 on store)
                    nc.vector.scalar_tensor_tensor(
                        h[c][m][:], ps[:], bcol(m, COL_B3),
                        h[c][m][:], op0=add, op1=add)

            def eg(c):
                # gate = sigmoid(wg.T @ ef + bg); the two m-halves are K=64
                # matmuls on distinct PE row groups so they run concurrently.
                efc = efs[c]
                psg = [psum_pool.tile([128, NS, S], f32, tag="ps",
                                      name=f"psg{m}") for m in range(2)]
                for j in range(NS):
                    for m in range(2):
                        nc.tensor.matmul(
                            psg[m][:, j],
                            wg[m * EVT:(m + 1) * EVT,
                               m * 128:(m + 1) * 128],
                            efc[m * EVT:(m + 1) * EVT, j],
                            start=True, stop=True,
                            tile_position=(64 * m, 0))
                gs = []
                for m in range(2):
                    gate = g_pool.tile([128, NS, S], f16, tag="g",
                                       name=f"g{c}_{m}")
                    nc.scalar.activation(gate[:], psg[m][:], Sigmoid,
                                         bias=bcol(m, COL_BG))
                    gs.append(gate)
                gates[c] = gs

            def eu(c):
                # u1 = relu(we1h.T @ h + we1e.T @ ef + be1), written e4m3
                # in DoubleRow layout [k=128, group=2(m), j, 512] for we2
                psu = [psum_pool.tile([128, NS, S], f32, tag="ps",
                                      name=f"psu{m}") for m in range(2)]
                for m in range(2):
                    for k in range(2):
                        wblk = we1h[k][:, m * 128:(m + 1) * 128]
                        for j in range(NS):
                            nc.tensor.matmul(psu[m][:, j], wblk,
                                             h[c][k][:, j],
                                             start=(k == 0), stop=False)
                for j in range(NS):
                    for m in range(2):
                        nc.tensor.matmul(
                            psu[m][:, j],
                            we1e[m * EVT:(m + 1) * EVT,
                                 m * 128:(m + 1) * 128],
                            efs[c][m * EVT:(m + 1) * EVT, j],
                            start=False, stop=True,
                            tile_position=(64 * m, 0))
                # relu commutes with positive scale: store u1/16 in e4m3 so
                # the x16 we2 weights cancel and psp is exactly We2 @ u1
                u1c = u_pool.tile([128, 2, NS, S], f8e4, tag="u",
                                  name=f"u{c}")
                for m in range(2):
                    nc.scalar.activation(u1c[:, m], psu[m][:], Relu,
                                         bias=bcol(m, COL_BE1S),
                                         scale=1.0 / WE2_SCALE)
                u1s[c] = u1c

            def ew(c):
                # upd = (16*We2).T @ (u1/16) (fp8 DoubleRow, K=256/matmul);
                # tmp = (psum + be2) * gate;  out = tmp + h.
                # The last chunk runs at j-granularity (512-wide ops) so the
                # post-matmul drain chain that ends the kernel is half as
                # long.
                u1c = u1s[c]
                jsplit = c == N_CHUNKS - 1
                for m in range(2):
                    for j0 in ([0, 1] if jsplit else [0]):
                        nj = 1 if jsplit else NS
                        psp = psum_pool.tile([128, nj, S], f32, tag="ps",
                                             name=f"psp{m}")
                        for j in range(nj):
                            nc.tensor.matmul(
                                psp[:, j],
                                we2dr[:, :, m * 128:(m + 1) * 128],
                                u1c[:, :, j0 + j, :],
                                start=True, stop=True, perf_mode=DR)
                        stg = stage_pool.tile([128, nj, S], f16, tag="st",
                                              name=f"s{c}_{m}")
                        tmp = t_pool.tile([128, nj, S], f16, tag="t",
                                          name=f"t{c}_{m}")
                        cols = slice(c * CHUNK + j0 * S,
                                     c * CHUNK + (j0 + nj) * S)
                        nc.vector.scalar_tensor_tensor(
                            tmp[:], psp[:], bcol(m, COL_BE2),
                            gates[c][m][:, j0:j0 + nj, :],
                            op0=add, op1=mult)
                        # split finals across DVE and the otherwise idle
                        # GpSimd so the end-of-kernel DVE backlog halves.
                        # Keep the last two chunks fully on DVE: its f16
                        # all-SBUF adds run at 2x (682ns) but only when
                        # GpSimd is idle -- a GpSimd op shares the POOL SBUF
                        # port with 2-read DVE ops and was measured slowing
                        # the final add 682 -> 2191ns.
                        eng = (nc.gpsimd if (m == 1 and c < N_CHUNKS - 2)
                               else nc.vector)
                        eng.tensor_add(stg[:], tmp[:],
                                       h[c][m][:, j0:j0 + nj, :])
                        nc.sync.dma_start(
                            outT_d.ap()[m * 128:(m + 1) * 128, cols],
                            stg[:])

            # ---- software-pipelined issue: per slot L1(c), L2(c-1),
            # L3(c-2); drains alternate ACT/ACT/DVE so the 4-tile psum pool
            # never gates the PE. Half the event sigmoids ride along in ODE
            # slots where ACT has slack. ----
            seq = [(s, c) for s in range(NUM_STEPS) for c in range(N_CHUNKS)]
            # all 8 event sigmoids ride in ODE slots (ACT has ~0.6us slack
            # per slot; one sigmoid per 4 slots stays under it cumulatively)
            eg_slots = {6 + 4 * k: k for k in range(N_CHUNKS)}
            for i, (s, c) in enumerate(seq):
                # eg first: its cheap matmuls run ahead of the slot's layer
                # matmuls, so the sigmoid queues before the slot's z-drains
                # on ACT and the eg psum frees ~2us earlier
                if i in eg_slots:
                    eg(eg_slots[i])
                l1(s, c)
                if i >= 1:
                    l2(*seq[i - 1])
                if i >= 2:
                    l3(*seq[i - 2])
            l2(*seq[-1])
            l3(*seq[-2])
            l3(*seq[-1])

            # ---- event pipeline, chunk-staggered ----
            for c in range(N_CHUNKS):
                eu(c)
                if c >= 1:
                    ew(c - 1)
            ew(N_CHUNKS - 1)

    nc.finalize()
    return nc


def _get_nc():
    if "nc" not in _CACHE:
        _CACHE["nc"] = _build()
    return _CACHE["nc"]


LAST_RESULT = None


def kernel(h_prev, event_features, t_start, t_end,
           Wt1, bt1, Wt2, bt2,
           Wd1, bd1, Wd2, bd2, Wd3, bd3,
           We1, be1, We2, be2, Wg, bg):
    global LAST_RESULT
    assert h_prev.shape == (B, HID) and event_features.shape == (B, EVT)

    # ---- host-side folding (float64 for exactness, cast down once) ----
    f8 = np.float64
    dt = (f8(t_end) - f8(t_start)) / NUM_STEPS
    b1s = np.empty((HID, NUM_STEPS), dtype=f8)
    for s in range(NUM_STEPS):
        t = f8(t_start) + s * dt
        te = np.tanh(t * Wt1[:, 0].astype(f8) + bt1.astype(f8))
        te = Wt2.astype(f8) @ te + bt2.astype(f8)
        b1s[:, s] = bd1.astype(f8) + Wd1[:, HID:].astype(f8) @ te

    xdt = np.float16
    w1T = np.ascontiguousarray(Wd1[:, :HID].T, dtype=xdt)
    w2T = np.ascontiguousarray(Wd2.T, dtype=xdt)
    w3T = np.ascontiguousarray((dt * Wd3.astype(f8)).T.astype(xdt))
    we1hT = np.ascontiguousarray(We1[:, :HID].T, dtype=xdt)
    we1eT = np.ascontiguousarray(We1[:, HID:].T, dtype=xdt)
    wgT = np.ascontiguousarray(Wg.T, dtype=xdt)
    # we2 DoubleRow: [k=128, group=2, m=256] = (16*We2.T)[k + 128*group, m]
    we2T16 = (WE2_SCALE * We2.astype(f8)).T.astype(np.float32)  # [256, 256]
    we2dr = np.ascontiguousarray(
        we2T16.reshape(2, 128, HID).transpose(1, 0, 2).reshape(128, 2 * HID)
    ).astype(E4NP)

    biasp = np.zeros((HID, N_BIAS_COLS), dtype=f8)
    biasp[:, COL_B1S:COL_B1S + NUM_STEPS] = b1s
    biasp[:, COL_B2] = bd2.astype(f8)
    biasp[:, COL_B3] = dt * bd3.astype(f8)
    biasp[:, COL_BE1S] = be1.astype(f8) / WE2_SCALE
    biasp[:, COL_BE2] = be2.astype(f8)
    biasp[:, COL_BG] = bg.astype(f8)
    biasp = biasp.astype(np.float32)

    hT = np.ascontiguousarray(h_prev.T, dtype=xdt)      # [HID, B]
    efT = np.ascontiguousarray(event_features.T, dtype=xdt)

    shared = dict(w1=w1T, w2=w2T, w3=w3T, we1h=we1hT, we1e=we1eT,
                  we2dr=we2dr, wg=wgT, biasp=biasp)
    in_maps = []
    for c in range(N_CORES):
        sl = slice(c * R, (c + 1) * R)
        in_maps.append(dict(
            hT=np.ascontiguousarray(hT[:, sl]),
            efT=np.ascontiguousarray(efT[:, sl]),
            **shared))

    nc = _get_nc()
    # First execution of a freshly-loaded NEFF occasionally faults the
    # exec unit (transient); retry recovers.
    last_err = None
    for _ in range(3):
        try:
            res = run_bass_kernel_spmd(nc, in_maps,
                                       core_ids=list(range(N_CORES)))
            break
        except Exception as e:  # noqa: BLE001
            last_err = e
            # a traced first execution can fault the exec unit; never trace
            # on retries
            os.environ["BASS_NEVER_TRACE"] = "1"
            import time
            time.sleep(2)
    else:
        raise last_err
    LAST_RESULT = res

    out = np.empty((B, HID), dtype=np.float32)
    for c in range(N_CORES):
        out[c * R:(c + 1) * R, :] = res.results[c]["outT"].T.astype(np.float32)
    return out


# revision 28
# speedup vs baseline: 1.0082x; 1.0082x over previous
"""EventDrivenODECell Trainium2 kernel.

Math (reference semantics):
  dt = (t_end - t_start)/5
  5 Euler steps: h += dt * (W3 tanh(W2 tanh(W1a h + [bd1 + W1b te(t)])) + bd3)
    where te(t) depends only on the scalar t -> folded on host into a
    per-step bias  b1s = bd1 + W1b @ te(t_s);  dt folded into W3/bd3.
  event: out = h + sigmoid(Wg ef + bg) * (We2 relu(We1h h + We1e ef + be1) + be2)

Device layout: feature-major activations [feat, batch]; batch sharded 8 ways
(8192 rows/core) processed in 8 column-chunks of 1024 rows. PSUM tiles are
[128, 2, 512] (2 banks); each N=512 matmul targets one bank while ACT/DVE
drain the pair in one 1024-wide op. Matmuls are float16 (full PE rate, FWL
weight loads, exact fp32 PSUM accumulation) except we2 which runs fp8e4
DoubleRow (K=256 in one instruction): its input u1 is written as e4m3 by the
relu and We2 is pre-scaled x16 on host so no weight lands in e4m3's
subnormal range; the relu stores u1/16 (relu commutes with positive scale,
via ACT's free input scale) so the x16 cancels inside the matmul and the
downstream ops are unchanged. Host-simulated rel err: 4.2e-3.
fp8 for the ODE/W1/W2/W3 matmuls was measured (host sim) at 2.2-2.4e-2 rel
err per weight tensor -- the quantization error is coherent across the 5
Euler steps -- so those stay f16.

Issue order is software-pipelined per chunk (L1(c), L2(c-1), L3(c-2)) so the
psum drains alternate ACT/ACT/DVE every slot instead of bunching per layer
sweep, and half the event sigmoids are hoisted into ODE slots where ACT has
slack. The event pipeline (gate/u1/we2) is chunk-staggered the same way.
gates/tmp/stage/outT are f16 (halves SBUF and store traffic).
"""

import os
import sys

sys.path.insert(0, "/opt/trn_rl_repo")

import numpy as np
import ml_dtypes

import concourse.bacc as bacc
import concourse.mybir as mybir
import concourse.tile as tile
from concourse.bass_utils import run_bass_kernel_spmd

B = 65536
HID = 256
EVT = 64
TEMB = 32
NUM_STEPS = 5
N_CORES = 8
R = B // N_CORES          # rows per core
S = 512                   # matmul moving-dim / PSUM bank quantum
CHUNK = 1024              # rows per processing chunk (= 2 PSUM banks)
NS = CHUNK // S           # N-splits per chunk
N_CHUNKS = R // CHUNK     # 8

WE2_SCALE = 16.0          # keeps We2 (|w| <= 1/16) out of e4m3 subnormals

MODE = "f16+we2fp8"

f32 = mybir.dt.float32
f16 = mybir.dt.float16
f8e4 = mybir.dt.float8e4
E4NP = ml_dtypes.float8_e4m3

_CACHE = {}

# bias-pack column indices
COL_B1S = 0          # 0..4: per-step layer-1 bias
COL_B2 = 5
COL_B3 = 6
COL_BE1S = 7         # be1 / 16 (the relu writes u1/16; see ew())
COL_BE2 = 8
COL_BG = 9
N_BIAS_COLS = 10


def _build():
    wdt = f16
    nc = bacc.Bacc("TRN2", target_bir_lowering=False, debug=False,
                   num_devices=N_CORES)

    hT_d = nc.dram_tensor("hT", [HID, R], wdt, kind="ExternalInput")
    efT_d = nc.dram_tensor("efT", [EVT, R], wdt, kind="ExternalInput")
    w1_d = nc.dram_tensor("w1", [HID, HID], wdt, kind="ExternalInput")
    w2_d = nc.dram_tensor("w2", [HID, HID], wdt, kind="ExternalInput")
    w3_d = nc.dram_tensor("w3", [HID, HID], wdt, kind="ExternalInput")
    we1h_d = nc.dram_tensor("we1h", [HID, HID], wdt, kind="ExternalInput")
    we1e_d = nc.dram_tensor("we1e", [EVT, HID], wdt, kind="ExternalInput")
    we2dr_d = nc.dram_tensor("we2dr", [128, 2 * HID], f8e4,
                             kind="ExternalInput")
    wg_d = nc.dram_tensor("wg", [EVT, HID], wdt, kind="ExternalInput")
    biasp_d = nc.dram_tensor("biasp", [HID, N_BIAS_COLS], f32,
                             kind="ExternalInput")
    outT_d = nc.dram_tensor("outT", [HID, R], f16, kind="ExternalOutput")

    Tanh = mybir.ActivationFunctionType.Tanh
    Sigmoid = mybir.ActivationFunctionType.Sigmoid
    Relu = mybir.ActivationFunctionType.Relu
    Ident = mybir.ActivationFunctionType.Identity
    add = mybir.AluOpType.add
    mult = mybir.AluOpType.mult
    DR = mybir.MatmulPerfMode.DoubleRow

    with tile.TileContext(nc) as tc:
        with (
            tc.tile_pool(name="consts", bufs=1) as consts,
            tc.tile_pool(name="h", bufs=1) as h_pool,
            tc.tile_pool(name="z1", bufs=12) as z1_pool,
            tc.tile_pool(name="z2", bufs=12) as z2_pool,
            tc.tile_pool(name="gts", bufs=16) as g_pool,
            tc.tile_pool(name="tmp", bufs=6) as t_pool,
            tc.tile_pool(name="u1", bufs=6) as u_pool,
            tc.tile_pool(name="efc", bufs=8) as ef_pool,
            tc.tile_pool(name="stage", bufs=6) as stage_pool,
            tc.tile_pool(name="psum", bufs=4, space="PSUM") as psum_pool,
        ):
            # ---- constants / h tiles; DMA issue order matters (single
            # HWDGE queue serializes) so interleave with first-use order ----
            def load_w(d, name, kparts, kdim=128):
                ts = []
                for k in range(kparts):
                    t = consts.tile([kdim, HID], wdt, tag=f"{name}{k}",
                                    name=f"{name}{k}")
                    nc.sync.dma_start(t[:], d.ap()[k * kdim:(k + 1) * kdim, :])
                    ts.append(t)
                return ts

            h = [[h_pool.tile([128, NS, S], wdt, tag=f"h{c}_{m}",
                              name=f"h{c}_{m}")
                  for m in range(2)] for c in range(N_CHUNKS)]

            def load_h(c):
                for m in range(2):
                    nc.sync.dma_start(
                        h[c][m][:],
                        hT_d.ap()[m * 128:(m + 1) * 128,
                                  c * CHUNK:(c + 1) * CHUNK])

            w1 = load_w(w1_d, "w1", 2)
            load_h(0)
            load_h(1)
            biasp = []
            for m in range(2):
                t = consts.tile([128, N_BIAS_COLS], f32, tag=f"biasp{m}",
                                name=f"biasp{m}")
                nc.sync.dma_start(t[:], biasp_d.ap()[m * 128:(m + 1) * 128, :])
                biasp.append(t)
            w2 = load_w(w2_d, "w2", 2)
            load_h(2)
            w3 = load_w(w3_d, "w3", 2)
            load_h(3)
            we1h = load_w(we1h_d, "we1h", 2)
            load_h(4)
            load_h(5)
            # we2 DoubleRow weights: [k=128, group=2, 256] e4m3, x16
            we2dr = consts.tile([128, 2, HID], f8e4, tag="we2dr",
                                name="we2dr")
            nc.sync.dma_start(we2dr[:], we2dr_d.ap())

            # EVT-dim weights live in both partition halves so the two
            # m-half K=64 matmuls can run on distinct PE row groups.
            def load_evt_w(d, name):
                t = consts.tile([128, HID], wdt, tag=name, name=name)
                nc.sync.dma_start(t[0:EVT, :], d.ap())
                nc.sync.dma_start(t[EVT:128, :], d.ap())
                return t

            we1e = load_evt_w(we1e_d, "we1e")   # [128, 256], duplicated rows
            wg = load_evt_w(wg_d, "wg")
            load_h(6)
            load_h(7)
            # event features early: DMA is idle mid-kernel and the event
            # phase must not wait on loads
            efs = {}
            for c in range(N_CHUNKS):
                efc = ef_pool.tile([128, NS, S], wdt, tag="ef",
                                   name=f"ef{c}")
                for half in range(2):
                    nc.sync.dma_start(
                        efc[half * EVT:(half + 1) * EVT],
                        efT_d.ap()[:, c * CHUNK:(c + 1) * CHUNK])
                efs[c] = efc

            def bcol(m, col):
                return biasp[m][:, col:col + 1]

            # ---- PE warmup: dependency-free junk matmuls ramp HAM to
            # full clock while the first h/w DMAs land. Measured variants:
            # memset-fed junk 16xN=256 beats real-matmul warmup (+4us: the
            # un-filled pipeline keeps the HAM busy-window from filling),
            # DMA-fed junk (+3us: the early DMA stream trickles), and
            # dependency-free N=1 const-AP junk (+3us: too sparse to fill
            # the busy-window). ----
            warm = consts.tile([128, 256], wdt, tag="warm", name="warm")
            nc.vector.memset(warm[:], 0.0)
            wps = psum_pool.tile([128, 256], f32, tag="ps", name="wps")
            for _ in range(16):
                nc.tensor.matmul(wps[:], warm[:, :128], warm[:],
                                 start=True, stop=True)
            # prefetch both ACT function tables while ACT is idle
            wz = stage_pool.tile([128, 256], f16, tag="wz", name="wz")
            nc.scalar.activation(wz[:], warm[:], Tanh)
            nc.scalar.activation(wz[:], warm[:], Sigmoid)

            def mm_chunk(ps, win, x, m, kparts=2, extra=None):
                """ps [128,NS,S] (PSUM) += win[k][:, m-blk].T @ x[k] per
                N-split; optional extra=(w_evt, ef_tile) accumulated last."""
                n_acc = kparts + (1 if extra is not None else 0)
                for k in range(kparts):
                    wblk = win[k][:, m * 128:(m + 1) * 128]
                    for j in range(NS):
                        nc.tensor.matmul(ps[:, j], wblk, x[k][:, j],
                                         start=(k == 0),
                                         stop=(k == n_acc - 1))
                if extra is not None:
                    ew, ex = extra
                    eblk = ew[:, m * 128:(m + 1) * 128]
                    for j in range(NS):
                        nc.tensor.matmul(ps[:, j], eblk, ex[:, j],
                                         start=False, stop=True)

            def dense(out_pool, win, x_tiles, bias_col, act, out_dt=wdt):
                """[2 x [128,NS,S]] tiles: act(win.T @ x + bias)."""
                outs = []
                for m in range(2):
                    ps = psum_pool.tile([128, NS, S], f32, tag="ps",
                                        name=f"ps{m}")
                    mm_chunk(ps, win, x_tiles, m)
                    o = out_pool.tile([128, NS, S], out_dt, tag="z",
                                      name=f"z{m}")
                    nc.scalar.activation(o[:], ps[:], act,
                                         bias=bcol(m, bias_col))
                    outs.append(o)
                return outs

            z1s = {}
            z2s = {}
            gates = {}
            u1s = {}

            def l1(s, c):
                z1s[c] = dense(z1_pool, w1, h[c], COL_B1S + s, Tanh)

            def l2(s, c):
                z2s[c] = dense(z2_pool, w2, z1s[c], COL_B2, Tanh)

            def l3(s, c):
                for m in range(2):
                    ps = psum_pool.tile([128, NS, S], f32, tag="ps",
                                        name=f"ps3{m}")
                    mm_chunk(ps, w3, z2s[c], m)
                    # h += (psum + b3)  (rounds h to f16 on store)
                    nc.vector.scalar_tensor_tensor(
                        h[c][m][:], ps[:], bcol(m, COL_B3),
                        h[c][m][:], op0=add, op1=add)

            def eg(c):
                # gate = sigmoid(wg.T @ ef + bg); the two m-halves are K=64
                # matmuls on distinct PE row groups so they run concurrently.
                efc = efs[c]
                psg = [psum_pool.tile([128, NS, S], f32, tag="ps",
                                      name=f"psg{m}") for m in range(2)]
                for j in range(NS):
                    for m in range(2):
                        nc.tensor.matmul(
                            psg[m][:, j],
                            wg[m * EVT:(m + 1) * EVT,
                               m * 128:(m + 1) * 128],
                            efc[m * EVT:(m + 1) * EVT, j],
                            start=True, stop=True,
                            tile_position=(64 * m, 0))
                gs = []
                for m in range(2):
                    gate = g_pool.tile([128, NS, S], f16, tag="g",
                                       name=f"g{c}_{m}")
                    nc.scalar.activation(gate[:], psg[m][:], Sigmoid,
                                         bias=bcol(m, COL_BG))
                    gs.append(gate)
                gates[c] = gs

            def eu(c):
                # u1 = relu(we1h.T @ h + we1e.T @ ef + be1), written e4m3
                # in DoubleRow layout [k=128, group=2(m), j, 512] for we2
                psu = [psum_pool.tile([128, NS, S], f32, tag="ps",
                                      name=f"psu{m}") for m in range(2)]
                for m in range(2):
                    for k in range(2):
                        wblk = we1h[k][:, m * 128:(m + 1) * 128]
                        for j in range(NS):
                            nc.tensor.matmul(psu[m][:, j], wblk,
                                             h[c][k][:, j],
                                             start=(k == 0), stop=False)
                for j in range(NS):
                    for m in range(2):
                        nc.tensor.matmul(
                            psu[m][:, j],
                            we1e[m * EVT:(m + 1) * EVT,
                                 m * 128:(m + 1) * 128],
                            efs[c][m * EVT:(m + 1) * EVT, j],
                            start=False, stop=True,
                            tile_position=(64 * m, 0))
                # relu commutes with positive scale: store u1/16 in e4m3 so
                # the x16 we2 weights cancel and psp is exactly We2 @ u1
                u1c = u_pool.tile([128, 2, NS, S], f8e4, tag="u",
                                  name=f"u{c}")
                for m in range(2):
                    nc.scalar.activation(u1c[:, m], psu[m][:], Relu,
                                         bias=bcol(m, COL_BE1S),
                                         scale=1.0 / WE2_SCALE)
                u1s[c] = u1c

            def ew(c):
                # upd = (16*We2).T @ (u1/16) (fp8 DoubleRow, K=256/matmul);
                # tmp = (psum + be2) * gate;  out = tmp + h.
                # The last chunk runs at j-granularity (512-wide ops) so the
                # post-matmul drain chain that ends the kernel is half as
                # long.
                u1c = u1s[c]
                for m in range(2):
                    psp = psum_pool.tile([128, NS, S], f32, tag="ps",
                                         name=f"psp{m}")
                    for j in range(NS):
                        nc.tensor.matmul(
                            psp[:, j],
                            we2dr[:, :, m * 128:(m + 1) * 128],
                            u1c[:, :, j, :],
                            start=True, stop=True, perf_mode=DR)
                    stg = stage_pool.tile([128, NS, S], f16, tag="st",
                                          name=f"s{c}_{m}")
                    tmp = t_pool.tile([128, NS, S], f16, tag="t",
                                      name=f"t{c}_{m}")
                    nc.vector.scalar_tensor_tensor(
                        tmp[:], psp[:], bcol(m, COL_BE2),
                        gates[c][m][:], op0=add, op1=mult)
                    # split finals across DVE and the otherwise idle GpSimd
                    # so the end-of-kernel DVE backlog halves. Keep the last
                    # two chunks fully on DVE: its f16 all-SBUF adds run at
                    # 2x (682ns) but only when GpSimd is idle -- a GpSimd op
                    # shares the POOL SBUF port with 2-read DVE ops and was
                    # measured slowing the final add 682 -> 2191ns.
                    eng = (nc.gpsimd if (m == 1 and c < N_CHUNKS - 2)
                           else nc.vector)
                    eng.tensor_add(stg[:], tmp[:], h[c][m][:])
                    nc.sync.dma_start(
                        outT_d.ap()[m * 128:(m + 1) * 128,
                                    c * CHUNK:(c + 1) * CHUNK],
                        stg[:])

            # ---- software-pipelined issue: per slot L1(c), L2(c-1),
            # L3(c-2); drains alternate ACT/ACT/DVE so the 4-tile psum pool
            # never gates the PE. Half the event sigmoids ride along in ODE
            # slots where ACT has slack. ----
            seq = [(s, c) for s in range(NUM_STEPS) for c in range(N_CHUNKS)]
            # all 8 event sigmoids ride in ODE slots (ACT has ~0.6us slack
            # per slot; one sigmoid per 4 slots stays under it cumulatively)
            eg_slots = {6 + 4 * k: k for k in range(N_CHUNKS)}
            for i, (s, c) in enumerate(seq):
                # eg first: its cheap matmuls run ahead of the slot's layer
                # matmuls, so the sigmoid queues before the slot's z-drains
                # on ACT and the eg psum frees ~2us earlier
                if i in eg_slots:
                    eg(eg_slots[i])
                l1(s, c)
                if i >= 1:
                    l2(*seq[i - 1])
                if i >= 2:
                    l3(*seq[i - 2])
            l2(*seq[-1])
            l3(*seq[-2])
            l3(*seq[-1])

            # ---- event pipeline, chunk-staggered ----
            for c in range(N_CHUNKS):
                eu(c)
                if c >= 1:
                    ew(c - 1)
            ew(N_CHUNKS - 1)

    nc.finalize()
    return nc


def _get_nc():
    if "nc" not in _CACHE:
        _CACHE["nc"] = _build()
    return _CACHE["nc"]


LAST_RESULT = None


def kernel(h_prev, event_features, t_start, t_end,
           Wt1, bt1, Wt2, bt2,
           Wd1, bd1, Wd2, bd2, Wd3, bd3,
           We1, be1, We2, be2, Wg, bg):
    global LAST_RESULT
    assert h_prev.shape == (B, HID) and event_features.shape == (B, EVT)

    # ---- host-side folding (float64 for exactness, cast down once) ----
    f8 = np.float64
    dt = (f8(t_end) - f8(t_start)) / NUM_STEPS
    b1s = np.empty((HID, NUM_STEPS), dtype=f8)
    for s in range(NUM_STEPS):
        t = f8(t_start) + s * dt
        te = np.tanh(t * Wt1[:, 0].astype(f8) + bt1.astype(f8))
        te = Wt2.astype(f8) @ te + bt2.astype(f8)
        b1s[:, s] = bd1.astype(f8) + Wd1[:, HID:].astype(f8) @ te

    xdt = np.float16
    w1T = np.ascontiguousarray(Wd1[:, :HID].T, dtype=xdt)
    w2T = np.ascontiguousarray(Wd2.T, dtype=xdt)
    w3T = np.ascontiguousarray((dt * Wd3.astype(f8)).T.astype(xdt))
    we1hT = np.ascontiguousarray(We1[:, :HID].T, dtype=xdt)
    we1eT = np.ascontiguousarray(We1[:, HID:].T, dtype=xdt)
    wgT = np.ascontiguousarray(Wg.T, dtype=xdt)
    # we2 DoubleRow: [k=128, group=2, m=256] = (16*We2.T)[k + 128*group, m]
    we2T16 = (WE2_SCALE * We2.astype(f8)).T.astype(np.float32)  # [256, 256]
    we2dr = np.ascontiguousarray(
        we2T16.reshape(2, 128, HID).transpose(1, 0, 2).reshape(128, 2 * HID)
    ).astype(E4NP)

    biasp = np.zeros((HID, N_BIAS_COLS), dtype=f8)
    biasp[:, COL_B1S:COL_B1S + NUM_STEPS] = b1s
    biasp[:, COL_B2] = bd2.astype(f8)
    biasp[:, COL_B3] = dt * bd3.astype(f8)
    biasp[:, COL_BE1S] = be1.astype(f8) / WE2_SCALE
    biasp[:, COL_BE2] = be2.astype(f8)
    biasp[:, COL_BG] = bg.astype(f8)
    biasp = biasp.astype(np.float32)

    hT = np.ascontiguousarray(h_prev.T, dtype=xdt)      # [HID, B]
    efT = np.ascontiguousarray(event_features.T, dtype=xdt)

    shared = dict(w1=w1T, w2=w2T, w3=w3T, we1h=we1hT, we1e=we1eT,
                  we2dr=we2dr, wg=wgT, biasp=biasp)
    in_maps = []
    for c in range(N_CORES):
        sl = slice(c * R, (c + 1) * R)
        in_maps.append(dict(
            hT=np.ascontiguousarray(hT[:, sl]),
            efT=np.ascontiguousarray(efT[:, sl]),
            **shared))

    nc = _get_nc()
    # First execution of a freshly-loaded NEFF occasionally faults the
    # exec unit (transient); retry recovers.
    last_err = None
    for _ in range(3):
        try:
            res = run_bass_kernel_spmd(nc, in_maps,
                                       core_ids=list(range(N_CORES)))
            break
        except Exception as e:  # noqa: BLE001
            last_err = e
            # a traced first execution can fault the exec unit; never trace
            # on retries
            os.environ["BASS_NEVER_TRACE"] = "1"
            import time
            time.sleep(2)
    else:
        raise last_err
    LAST_RESULT = res

    out = np.empty((B, HID), dtype=np.float32)
    for c in range(N_CORES):
        out[c * R:(c + 1) * R, :] = res.results[c]["outT"].T.astype(np.float32)
    return out
